# revision 14
# baseline (speedup 1.0000x reference)
"""Trainium2 Bass kernel for nn_EquivariantProductBasisBlock (MACE symmetric
contraction, correlation 3, irreps 0e+1o -> 0e+1o, + e3nn linear).

Strategy (data-parallel over nodes, 8 cores):
  Per core: 64 nodes x 64 channels = 4096 (b,c) pairs, each with a 9-dim
  feature vector x.  The full contraction reduces to, per pair:
      T[(D,q)] = sum_f  F[f] * Ucat[f, (D,q)]          (matmul, f = 219)
      f[D]     = sum_q  Wexp[(D,q)] * T[(D,q)]          (species weights)
      out      = blockdiag(Wlin) applied over channels  (matmul)
  where F = [x (9) | sym pairs x_j x_k (45) | sym triples x_i x_j x_k (165)]
  and Ucat folds the (symmetric) U3/U2/U1 CG tensors with permutation
  multiplicities.  Species gather + all weight packing + all data-layout
  shuffles happen host-side so every DMA is contiguous.

Device pipeline per core:
  DMA x -> DVE monomials (natural layout [bc, f]) -> transpose to [f, bc]
  (PE identity-transpose + evac, or DMA xbar transpose) -> PE matmul vs
  Ucat -> DVE * Wexp -> PE segmented-sum -> PE blockdiag Wlin -> DMA out.
"""

import os
import sys

for _p in ("/opt/trn_rl_repo",):
    if _p not in sys.path:
        sys.path.insert(0, _p)

import numpy as np
import ml_dtypes

N_CORES = 8
N_NODES = 512
B = N_NODES // N_CORES  # nodes per core
C = 64                  # channels
NF = 9                  # features per channel
BC = B * C              # 4096 pairs per core
G = BC // 128           # 32 partition tiles
K3, K2, K1 = 16, 4, 1
NQ = K3 + K2 + K1       # 21
ND = 4                  # output dims: idx0 d=1, idx1 d=3
MUL = 64

# Symmetric bases ------------------------------------------------------------
PAIRS = [(j, k) for j in range(NF) for k in range(j, NF)]  # 45, j<=k
TRI2 = {jk: t for t, jk in enumerate(PAIRS)}
NP2 = len(PAIRS)  # 45
SEG_OFF = []
SEG_LEN = []
_off = 0
for i in range(NF):
    SEG_OFF.append(_off)
    SEG_LEN.append(NP2 - TRI2[(i, i)])
    _off += SEG_LEN[-1]
NP3 = _off  # 165
NFEAT_TOT = NF + NP2 + NP3  # 219
# two overlapping 128-row chunks: [0,128) and [91,219); the overlap rows
# (91..127) are zeroed in the second U chunk so nothing double-counts.
CH0_LO, CH0_HI = 0, 128
CH1_LO, CH1_HI = NFEAT_TOT - 128, NFEAT_TOT  # 91..219
OVL = CH0_HI - CH1_LO  # 37

F_COL_X = 0
F_COL_P2 = NF          # 9
F_COL_P3 = NF + NP2    # 54

BF16 = ml_dtypes.bfloat16

# "dma": xbar DMA transpose (no PSUM, no evac); "pe": PE identity transpose
TRANSPOSE_MODE = os.environ.get("K_TRANSPOSE", "dma")

_CACHE = {}


def _mult3(i, j, k):
    if i == j == k:
        return 1.0
    if i == j or j == k or i == k:
        return 3.0
    return 6.0


def _host_pack(node_feats, node_specie,
               U3_0, U2_0, U1_0, w3_0, w2_0, w1_0,
               U3_1, U2_1, U1_1, w3_1, w2_1, w1_1,
               Wlin0, Wlin1):
    node_feats = np.asarray(node_feats, np.float32)
    spec = np.asarray(node_specie).astype(np.int64)

    # --- Ucat [219, 84] ---
    ucat = np.zeros((NFEAT_TOT, ND * NQ), np.float32)
    Us = [(np.asarray(U3_0, np.float32), np.asarray(U2_0, np.float32),
           np.asarray(U1_0, np.float32)),
          (np.asarray(U3_1, np.float32), np.asarray(U2_1, np.float32),
           np.asarray(U1_1, np.float32))]
    for D in range(ND):
        idx = 0 if D == 0 else 1
        d = 0 if D == 0 else D - 1
        U3, U2, U1 = Us[idx]
        col = D * NQ
        ucat[F_COL_X:F_COL_X + NF, col + K3 + K2] = U1[d, :, 0]
        for t, (j, k) in enumerate(PAIRS):
            m2 = 1.0 if j == k else 2.0
            ucat[F_COL_P2 + t, col + K3:col + K3 + K2] = m2 * U2[d, j, k, :]
        for i in range(NF):
            for s, (j, k) in enumerate(PAIRS[TRI2[(i, i)]:]):
                r = F_COL_P3 + SEG_OFF[i] + s
                ucat[r, col:col + K3] = _mult3(i, j, k) * U3[d, i, j, k, :]
    u0 = ucat[CH0_LO:CH0_HI].copy()
    u1 = ucat[CH1_LO:CH1_HI].copy()
    u1[0:OVL] = 0.0

    # --- per-node species weights, transposed layout [84, b, c] ---
    wcat = np.concatenate([
        np.asarray(w3_0, np.float32), np.asarray(w2_0, np.float32),
        np.asarray(w1_0, np.float32), np.asarray(w3_1, np.float32),
        np.asarray(w2_1, np.float32), np.asarray(w1_1, np.float32),
    ], axis=1)                      # [NSPEC, 42, C]
    wnode = wcat[spec]              # [512, 42, C]

    # --- segment-sum selector [84, 4] ---
    sseg = np.zeros((ND * NQ, ND), np.float32)
    for D in range(ND):
        sseg[D * NQ:(D + 1) * NQ, D] = 1.0

    # --- block-diag Wlin [2, 128, 128] (path norm 1/sqrt(C) folded in) ---
    inv_sqrt_c = 1.0 / np.sqrt(np.float32(C))
    bw = np.zeros((2, 128, 128), np.float32)
    for b2 in range(2):
        bw[0, b2 * 64:(b2 + 1) * 64, b2 * 64:(b2 + 1) * 64] = \
            np.asarray(Wlin0, np.float32) * inv_sqrt_c
        bw[1, b2 * 64:(b2 + 1) * 64, b2 * 64:(b2 + 1) * 64] = \
            np.asarray(Wlin1, np.float32) * inv_sqrt_c

    ident = np.eye(128, dtype=np.float32)

    # one [128, 552] bf16 blob: u0 | u1 | bw0 | bw1 | ident
    cblob = np.zeros((128, 552), np.float32)
    cblob[:, 0:84] = u0
    cblob[:, 84:168] = u1
    cblob[:, 168:296] = bw[0]
    cblob[:, 296:424] = bw[1]
    cblob[:, 424:552] = ident
    cblob = cblob.astype(BF16)

    in_maps = []
    for core in range(N_CORES):
        b0 = core * B
        # x pre-shuffled to device layout [128=(b2,c), g, i], contiguous
        xs = node_feats[b0:b0 + B].reshape(G, 2, C, NF)      # [g, b2, c, i]
        xs = np.ascontiguousarray(xs.transpose(1, 2, 0, 3))  # [b2, c, g, i]
        xs = xs.reshape(128, G, NF)
        wex42 = wnode[b0:b0 + B].transpose(1, 0, 2)          # [42, B, C]
        wex84 = np.concatenate(
            [wex42[0:NQ]] + [wex42[NQ:2 * NQ]] * 3, axis=0)  # [84, B, C]
        # [84, 4100] bf16 blob: wexp | sseg
        wblob = np.zeros((ND * NQ, BC + ND), np.float32)
        wblob[:, 0:BC] = wex84.reshape(ND * NQ, BC)
        wblob[:, BC:BC + ND] = sseg
        in_maps.append({
            "x": xs,
            "cblob": cblob,
            "wblob": wblob.astype(BF16),
        })
    return in_maps


def _host_unpack(res):
    """Device returns o [128=(b2,M), 128] per core; reassemble [512, 256]."""
    out = np.zeros((N_NODES, ND * MUL), np.float32)
    for core in range(N_CORES):
        o = res[core]["o"]                       # [128, 128]
        o = o.reshape(2, MUL, 128)               # [b2, M, col]
        b0 = core * B
        # col 0..31 = g (D0);  col 32.. = (g, i)
        o0 = o[:, :, 0:G]                        # [b2, M, g]
        o1 = o[:, :, G:G + 3 * G].reshape(2, MUL, G, 3)
        for b2 in range(2):
            rows = b0 + 2 * np.arange(G) + b2    # [g]
            out[rows, 0:MUL] = o0[b2].T          # [g, M]
            cols = (MUL + 3 * np.arange(MUL)[None, :, None]
                    + np.arange(3)[None, None, :])      # [1, M, 3]
            out[rows[:, None, None], cols] = o1[b2].transpose(1, 0, 2)
    return out


def _build_nc():
    import concourse.bass as bass
    import concourse.tile as tile
    from concourse import mybir, bacc

    F32 = mybir.dt.float32
    BF = mybir.dt.bfloat16

    nc = bacc.Bacc("TRN2", target_bir_lowering=False, debug=False,
                   num_devices=N_CORES)

    x_d = nc.dram_tensor("x", [128, G, NF], F32, kind="ExternalInput").ap()
    cblob_d = nc.dram_tensor("cblob", [128, 552], BF,
                             kind="ExternalInput").ap()
    wblob_d = nc.dram_tensor("wblob", [ND * NQ, BC + ND], BF,
                             kind="ExternalInput").ap()
    o_d = nc.dram_tensor("o", [128, 128], F32, kind="ExternalOutput").ap()

    # g-tiles per transpose/evac batch: one PSUM bank each
    EB = 8 if TRANSPOSE_MODE == "pe" else 4

    with tile.TileContext(nc) as tc:
        with (
            tc.tile_pool(name="const", bufs=1) as constp,
            tc.tile_pool(name="xin", bufs=1) as xinp,
            tc.tile_pool(name="fnat", bufs=1) as fnatp,
            tc.tile_pool(name="ft", bufs=1) as ftp,
            tc.tile_pool(name="gbuf", bufs=1) as gbufp,
            tc.tile_pool(name="fsb", bufs=1) as fsbp,
            tc.tile_pool(name="tps", bufs=2, space="PSUM") as tpsp,
            tc.tile_pool(name="fps", bufs=1, space="PSUM") as fpsp,
            tc.tile_pool(name="ops", bufs=1, space="PSUM") as opsp,
        ):
            # ---- constants in (2 blob DMAs, split across HWDGE queues) ----
            cb_sb = constp.tile([128, 552], BF)
            nc.sync.dma_start(cb_sb[:], cblob_d)
            wb_sb = constp.tile([ND * NQ, BC + ND], BF)
            nc.scalar.dma_start(wb_sb[:], wblob_d)
            u0_sb = cb_sb[:, 0:84]
            u1_sb = cb_sb[:, 84:168]
            bw0_sb = cb_sb[:, 168:296]
            bw1_sb = cb_sb[:, 296:424]
            id_sb = cb_sb[:, 424:552]
            wexp_sb = wb_sb[:ND * NQ, 0:BC]
            sseg_sb = wb_sb[:ND * NQ, BC:BC + ND]

            x_all = xinp.tile([128, G, NF], F32)
            nc.sync.dma_start(x_all[:], x_d)

            f_nat = fnatp.tile([128, G, NFEAT_TOT], BF)
            y2 = xinp.tile([128, G, NP2], F32)

            # ---- PE warmup: keep HAM at K=8/8 while DVE forms monomials ----
            N_WARM = int(os.environ.get("K_WARM", "26"))
            warm_ps = None
            if N_WARM:
                warm_ps = opsp.tile([128, 512], F32, tag="ops", name="warm")
                for w in range(N_WARM):
                    nc.tensor.matmul(warm_ps[:], id_sb, cb_sb[:, 0:512],
                                     start=True, stop=True)

            ft0 = ftp.tile([128, BC], BF)
            ft1 = ftp.tile([128, BC], BF)
            NSPLIT = 2
            GH = G // NSPLIT
            GPS = os.environ.get("K_GPSIMD", "1") == "1"

            with (
                tc.tile_pool(name="tp0", bufs=2, space="PSUM") as tp0p,
                tc.tile_pool(name="tp1", bufs=2, space="PSUM") as tp1p,
            ):
                for h in range(NSPLIT):
                    gs = slice(h * GH, (h + 1) * GH)
                    xs = x_all[:, gs]
                    # ---- monomials for this half ----
                    nc.scalar.copy(f_nat[:, gs, F_COL_X:F_COL_X + NF], xs)
                    for j in range(NF):
                        n = NF - j
                        t0 = TRI2[(j, j)]
                        eng = nc.gpsimd if (GPS and j < 2) else nc.vector
                        eng.tensor_mul(
                            y2[:, gs, t0:t0 + n],
                            xs[:, :, j:j + 1].broadcast_to([128, GH, n]),
                            xs[:, :, j:NF])
                    nc.scalar.copy(f_nat[:, gs, F_COL_P2:F_COL_P2 + NP2],
                                   y2[:, gs])
                    for i in range(NF):
                        n = SEG_LEN[i]
                        t0 = TRI2[(i, i)]
                        o = F_COL_P3 + SEG_OFF[i]
                        eng = nc.gpsimd if (GPS and i == 0) else nc.vector
                        eng.tensor_mul(
                            f_nat[:, gs, o:o + n],
                            xs[:, :, i:i + 1].broadcast_to([128, GH, n]),
                            y2[:, gs, t0:NP2])

                    # ---- transpose this half's tiles to [f, bc] ----
                    if TRANSPOSE_MODE == "dma":
                        for g in range(h * GH, (h + 1) * GH):
                            nc.sync.dma_start_transpose(
                                ft0[:, g * 128:(g + 1) * 128],
                                f_nat[:, g, CH0_LO:CH0_HI])
                            nc.sync.dma_start_transpose(
                                ft1[:, g * 128:(g + 1) * 128],
                                f_nat[:, g, CH1_LO:CH1_HI])
                        continue
                    use_tmode = TRANSPOSE_MODE == "pe"
                    for bi in range(h * (GH // EB), (h + 1) * (GH // EB)):
                        p0 = tp0p.tile([128, EB, 128],
                                       BF if use_tmode else F32, tag="tp0")
                        p1 = tp1p.tile([128, EB, 128],
                                       BF if use_tmode else F32, tag="tp1")
                        for e in range(EB):
                            g = bi * EB + e
                            if use_tmode:
                                nc.tensor.transpose(
                                    p0[:, e], f_nat[:, g, CH0_LO:CH0_HI],
                                    id_sb)
                                nc.tensor.transpose(
                                    p1[:, e], f_nat[:, g, CH1_LO:CH1_HI],
                                    id_sb)
                            else:
                                nc.tensor.matmul(
                                    p0[:, e], f_nat[:, g, CH0_LO:CH0_HI],
                                    id_sb, start=True, stop=True)
                                nc.tensor.matmul(
                                    p1[:, e], f_nat[:, g, CH1_LO:CH1_HI],
                                    id_sb, start=True, stop=True)
                        cols = slice(bi * EB * 128, (bi + 1) * EB * 128)
                        nc.scalar.copy(ft0[:, cols], p0[:])
                        nc.scalar.copy(ft1[:, cols], p1[:])

            # ---- main contraction + species weights ----
            gb = gbufp.tile([ND * NQ, BC], BF)
            NGRP = 4 if TRANSPOSE_MODE == "dma" else 8
            W = BC // NGRP
            for n in range(NGRP):
                cols = slice(n * W, (n + 1) * W)
                t_ps = tpsp.tile([ND * NQ, W], F32, tag="tps")
                for m in range(W // 512):
                    mc = slice(m * 512, (m + 1) * 512)
                    fc = slice(n * W + m * 512, n * W + (m + 1) * 512)
                    nc.tensor.matmul(t_ps[:, mc], u0_sb, ft0[:, fc],
                                     start=True, stop=False)
                    nc.tensor.matmul(t_ps[:, mc], u1_sb, ft1[:, fc],
                                     start=False, stop=True)
                nc.vector.tensor_mul(gb[:, cols], wexp_sb[:, cols], t_ps[:])

            # ---- segmented sum over q (PE): f[bc, D] ----
            f_ps = fpsp.tile([128, G, ND], F32)
            for g in range(G):
                nc.tensor.matmul(f_ps[:, g], gb[:, g * 128:(g + 1) * 128],
                                 sseg_sb, start=True, stop=True)
            f_sb = fsbp.tile([128, G, ND], BF)
            nc.scalar.copy(f_sb[:], f_ps[:])

            # ---- final linear (block-diag Wlin over channels) ----
            o_ps = opsp.tile([128, 128], F32, tag="ops")
            nc.tensor.matmul(o_ps[:, 0:G], bw0_sb, f_sb[:, :, 0],
                             start=True, stop=True)
            nc.tensor.matmul(
                o_ps[:, G:G + G * 3].rearrange("p (g i) -> p g i", g=G),
                bw1_sb, f_sb[:, :, 1:4], start=True, stop=True)

            # ---- output (contiguous; host unshuffles) ----
            o_sb = fsbp.tile([128, 128], F32)
            nc.vector.tensor_copy(o_sb[:], o_ps[:])
            nc.sync.dma_start(o_d, o_sb[:])

    nc.compile()
    return nc


def _get_nc():
    if "nc" not in _CACHE:
        _CACHE["nc"] = _build_nc()
    return _CACHE["nc"]


def kernel(node_feats, node_specie,
           U3_0, U2_0, U1_0, w3_0, w2_0, w1_0,
           U3_1, U2_1, U1_1, w3_1, w2_1, w1_1,
           Wlin0, Wlin1):
    from concourse.bass_utils import run_bass_kernel_spmd

    in_maps = _host_pack(node_feats, node_specie,
                         U3_0, U2_0, U1_0, w3_0, w2_0, w1_0,
                         U3_1, U2_1, U1_1, w3_1, w2_1, w1_1,
                         Wlin0, Wlin1)
    nc = _get_nc()
    res = run_bass_kernel_spmd(nc, in_maps, core_ids=list(range(N_CORES)))
    return _host_unpack(res.results).astype(np.float32)


# revision 17
# speedup vs baseline: 1.1208x; 1.1208x over previous
"""Trainium2 Bass kernel for nn_EquivariantProductBasisBlock (MACE symmetric
contraction, correlation 3, irreps 0e+1o -> 0e+1o, + e3nn linear).

Strategy (data-parallel over nodes, 8 cores):
  Per core: 64 nodes x 64 channels = 4096 (b,c) pairs, each with a 9-dim
  feature vector x.  The full contraction reduces to, per pair:
      T[(D,q)] = sum_f  F[f] * Ucat[f, (D,q)]          (matmul, f = 219)
      f[D]     = sum_q  Wexp[(D,q)] * T[(D,q)]          (species weights)
      out      = blockdiag(Wlin) applied over channels  (matmul)
  where F = [x (9) | sym pairs x_j x_k (45) | sym triples x_i x_j x_k (165)]
  and Ucat folds the (symmetric) U3/U2/U1 CG tensors with permutation
  multiplicities.  Species gather + all weight packing + all data-layout
  shuffles happen host-side so every DMA is contiguous.

Device pipeline per core:
  DMA x -> DVE monomials (natural layout [bc, f]) -> transpose to [f, bc]
  (PE identity-transpose + evac, or DMA xbar transpose) -> PE matmul vs
  Ucat -> DVE * Wexp -> PE segmented-sum -> PE blockdiag Wlin -> DMA out.
"""

import os
import sys

for _p in ("/opt/trn_rl_repo",):
    if _p not in sys.path:
        sys.path.insert(0, _p)

import numpy as np
import ml_dtypes

N_CORES = 8
N_NODES = 512
B = N_NODES // N_CORES  # nodes per core
C = 64                  # channels
NF = 9                  # features per channel
BC = B * C              # 4096 pairs per core
G = BC // 128           # 32 partition tiles
K3, K2, K1 = 16, 4, 1
NQ = K3 + K2 + K1       # 21
ND = 4                  # output dims: idx0 d=1, idx1 d=3
MUL = 64

# Symmetric bases ------------------------------------------------------------
PAIRS = [(j, k) for j in range(NF) for k in range(j, NF)]  # 45, j<=k
TRI2 = {jk: t for t, jk in enumerate(PAIRS)}
NP2 = len(PAIRS)  # 45
SEG_OFF = []
SEG_LEN = []
_off = 0
for i in range(NF):
    SEG_OFF.append(_off)
    SEG_LEN.append(NP2 - TRI2[(i, i)])
    _off += SEG_LEN[-1]
NP3 = _off  # 165
NFEAT_TOT = NF + NP2 + NP3  # 219
# two overlapping 128-row chunks: [0,128) and [91,219); the overlap rows
# (91..127) are zeroed in the second U chunk so nothing double-counts.
CH0_LO, CH0_HI = 0, 128
CH1_LO, CH1_HI = 128, NFEAT_TOT  # 91 rows
CH1_N = CH1_HI - CH1_LO

F_COL_X = 0
F_COL_P2 = NF          # 9
F_COL_P3 = NF + NP2    # 54

BF16 = ml_dtypes.bfloat16

# "dma": xbar DMA transpose (no PSUM, no evac); "pe": PE identity transpose
TRANSPOSE_MODE = os.environ.get("K_TRANSPOSE", "dma")

_CACHE = {}


def _mult3(i, j, k):
    if i == j == k:
        return 1.0
    if i == j or j == k or i == k:
        return 3.0
    return 6.0


def _host_pack(node_feats, node_specie,
               U3_0, U2_0, U1_0, w3_0, w2_0, w1_0,
               U3_1, U2_1, U1_1, w3_1, w2_1, w1_1,
               Wlin0, Wlin1):
    node_feats = np.asarray(node_feats, np.float32)
    spec = np.asarray(node_specie).astype(np.int64)

    # --- Ucat [219, 84] ---
    ucat = np.zeros((NFEAT_TOT, ND * NQ), np.float32)
    Us = [(np.asarray(U3_0, np.float32), np.asarray(U2_0, np.float32),
           np.asarray(U1_0, np.float32)),
          (np.asarray(U3_1, np.float32), np.asarray(U2_1, np.float32),
           np.asarray(U1_1, np.float32))]
    for D in range(ND):
        idx = 0 if D == 0 else 1
        d = 0 if D == 0 else D - 1
        U3, U2, U1 = Us[idx]
        col = D * NQ
        ucat[F_COL_X:F_COL_X + NF, col + K3 + K2] = U1[d, :, 0]
        for t, (j, k) in enumerate(PAIRS):
            m2 = 1.0 if j == k else 2.0
            ucat[F_COL_P2 + t, col + K3:col + K3 + K2] = m2 * U2[d, j, k, :]
        for i in range(NF):
            for s, (j, k) in enumerate(PAIRS[TRI2[(i, i)]:]):
                r = F_COL_P3 + SEG_OFF[i] + s
                ucat[r, col:col + K3] = _mult3(i, j, k) * U3[d, i, j, k, :]
    u0 = ucat[CH0_LO:CH0_HI].copy()
    u1 = ucat[CH1_LO:CH1_HI].copy()

    # --- per-node species weights, transposed layout [84, b, c] ---
    wcat = np.concatenate([
        np.asarray(w3_0, np.float32), np.asarray(w2_0, np.float32),
        np.asarray(w1_0, np.float32), np.asarray(w3_1, np.float32),
        np.asarray(w2_1, np.float32), np.asarray(w1_1, np.float32),
    ], axis=1)                      # [NSPEC, 42, C]
    wnode = wcat[spec]              # [512, 42, C]

    # --- segment-sum selector [84, 4] ---
    sseg = np.zeros((ND * NQ, ND), np.float32)
    for D in range(ND):
        sseg[D * NQ:(D + 1) * NQ, D] = 1.0

    # --- block-diag Wlin [2, 128, 128] (path norm 1/sqrt(C) folded in) ---
    inv_sqrt_c = 1.0 / np.sqrt(np.float32(C))
    bw = np.zeros((2, 128, 128), np.float32)
    for b2 in range(2):
        bw[0, b2 * 64:(b2 + 1) * 64, b2 * 64:(b2 + 1) * 64] = \
            np.asarray(Wlin0, np.float32) * inv_sqrt_c
        bw[1, b2 * 64:(b2 + 1) * 64, b2 * 64:(b2 + 1) * 64] = \
            np.asarray(Wlin1, np.float32) * inv_sqrt_c

    ident = np.eye(128, dtype=np.float32)

    # one [128, 552] bf16 blob: u0 | u1 | bw0 | bw1 | ident
    cblob = np.zeros((128, 552), np.float32)
    cblob[:, 0:84] = u0
    cblob[0:91, 84:168] = u1
    cblob[:, 168:296] = bw[0]
    cblob[:, 296:424] = bw[1]
    cblob[:, 424:552] = ident
    cblob = cblob.astype(BF16)

    in_maps = []
    for core in range(N_CORES):
        b0 = core * B
        # x pre-shuffled to device layout [128=(b2,c), g, i], contiguous
        xs = node_feats[b0:b0 + B].reshape(G, 2, C, NF)      # [g, b2, c, i]
        xs = np.ascontiguousarray(xs.transpose(1, 2, 0, 3))  # [b2, c, g, i]
        xs = xs.reshape(128, G, NF)
        wex42 = wnode[b0:b0 + B].transpose(1, 0, 2)          # [42, B, C]
        wex84 = np.concatenate(
            [wex42[0:NQ]] + [wex42[NQ:2 * NQ]] * 3, axis=0)  # [84, B, C]
        # [84, 4100] bf16 blob: wexp | sseg
        wblob = np.zeros((ND * NQ, BC + ND), np.float32)
        wblob[:, 0:BC] = wex84.reshape(ND * NQ, BC)
        wblob[:, BC:BC + ND] = sseg
        in_maps.append({
            "x": xs,
            "cblob": cblob,
            "wblob": wblob.astype(BF16),
        })
    return in_maps


def _host_unpack(res):
    """Device returns o [128=(b2,M), 128] per core; reassemble [512, 256]."""
    out = np.zeros((N_NODES, ND * MUL), np.float32)
    for core in range(N_CORES):
        o = res[core]["o"]                       # [128, 128]
        o = o.reshape(2, MUL, 128)               # [b2, M, col]
        b0 = core * B
        # col 0..31 = g (D0);  col 32.. = (g, i)
        o0 = o[:, :, 0:G]                        # [b2, M, g]
        o1 = o[:, :, G:G + 3 * G].reshape(2, MUL, G, 3)
        for b2 in range(2):
            rows = b0 + 2 * np.arange(G) + b2    # [g]
            out[rows, 0:MUL] = o0[b2].T          # [g, M]
            cols = (MUL + 3 * np.arange(MUL)[None, :, None]
                    + np.arange(3)[None, None, :])      # [1, M, 3]
            out[rows[:, None, None], cols] = o1[b2].transpose(1, 0, 2)
    return out


def _build_nc():
    import concourse.bass as bass
    import concourse.tile as tile
    from concourse import mybir, bacc

    F32 = mybir.dt.float32
    BF = mybir.dt.bfloat16

    nc = bacc.Bacc("TRN2", target_bir_lowering=False, debug=False,
                   num_devices=N_CORES)

    x_d = nc.dram_tensor("x", [128, G, NF], F32, kind="ExternalInput").ap()
    cblob_d = nc.dram_tensor("cblob", [128, 552], BF,
                             kind="ExternalInput").ap()
    wblob_d = nc.dram_tensor("wblob", [ND * NQ, BC + ND], BF,
                             kind="ExternalInput").ap()
    o_d = nc.dram_tensor("o", [128, 128], F32, kind="ExternalOutput").ap()

    # g-tiles per transpose/evac batch: one PSUM bank each
    EB = 8 if TRANSPOSE_MODE == "pe" else 4

    with tile.TileContext(nc) as tc:
        with (
            tc.tile_pool(name="const", bufs=1) as constp,
            tc.tile_pool(name="xin", bufs=1) as xinp,
            tc.tile_pool(name="fnat", bufs=1) as fnatp,
            tc.tile_pool(name="ft", bufs=1) as ftp,
            tc.tile_pool(name="gbuf", bufs=1) as gbufp,
            tc.tile_pool(name="fsb", bufs=1) as fsbp,
            tc.tile_pool(name="tps", bufs=2, space="PSUM") as tpsp,
            tc.tile_pool(name="fps", bufs=1, space="PSUM") as fpsp,
            tc.tile_pool(name="ops", bufs=1, space="PSUM") as opsp,
        ):
            # ---- inputs (x first: it gates the whole pipeline) ----
            x_all = xinp.tile([128, G, NF], F32)
            nc.sync.dma_start(x_all[:, 0:G // 2], x_d[:, 0:G // 2])
            nc.scalar.dma_start(x_all[:, G // 2:G], x_d[:, G // 2:G])
            cb_sb = constp.tile([128, 552], BF)
            nc.sync.dma_start(cb_sb[:], cblob_d)
            wb_sb = constp.tile([ND * NQ, BC + ND], BF)
            nc.scalar.dma_start(wb_sb[:], wblob_d)
            u0_sb = cb_sb[:, 0:84]
            u1_sb = cb_sb[0:91, 84:168]
            bw0_sb = cb_sb[:, 168:296]
            bw1_sb = cb_sb[:, 296:424]
            id_sb = cb_sb[:, 424:552]
            wexp_sb = wb_sb[:ND * NQ, 0:BC]
            sseg_sb = wb_sb[:ND * NQ, BC:BC + ND]

            f_nat = fnatp.tile([128, G, NFEAT_TOT], BF)
            y2 = xinp.tile([128, G, NP2], F32)

            # ---- monomial formation (x/pair casts on ScalarE) ----
            nc.scalar.copy(f_nat[:, :, F_COL_X:F_COL_X + NF], x_all[:])

            # PE warmup: gated on the x-cast so the HAM window flips to
            # K=8/8 right as the transposes become ready.
            N_WARM = int(os.environ.get("K_WARM", "20"))
            if N_WARM:
                warm_ps = opsp.tile([128, 512], F32, tag="ops", name="warm")
                wrhs = f_nat[:, :, F_COL_X:F_COL_X + NF]
                for w in range(N_WARM):
                    nc.tensor.matmul(warm_ps[:, 0:G * NF], id_sb, wrhs,
                                     start=True, stop=True)

            for j in range(NF):
                n = NF - j
                t0 = TRI2[(j, j)]
                nc.vector.tensor_mul(
                    y2[:, :, t0:t0 + n],
                    x_all[:, :, j:j + 1].broadcast_to([128, G, n]),
                    x_all[:, :, j:NF])
            nc.scalar.copy(f_nat[:, :, F_COL_P2:F_COL_P2 + NP2], y2[:])
            for i in range(NF):
                n = SEG_LEN[i]
                t0 = TRI2[(i, i)]
                o = F_COL_P3 + SEG_OFF[i]
                nc.vector.tensor_mul(
                    f_nat[:, :, o:o + n],
                    x_all[:, :, i:i + 1].broadcast_to([128, G, n]),
                    y2[:, :, t0:NP2])

            # ---- transpose F to [f, bc] ----
            ft0 = ftp.tile([128, BC], BF)
            ft1 = ftp.tile([CH1_N, BC], BF)
            use_tmode = TRANSPOSE_MODE == "pe"
            with (
                tc.tile_pool(name="tp0", bufs=2, space="PSUM") as tp0p,
                tc.tile_pool(name="tp1", bufs=2, space="PSUM") as tp1p,
            ):
                for bi in range(G // EB):
                    p0 = tp0p.tile([128, EB, 128],
                                   BF if use_tmode else F32, tag="tp0")
                    p1 = tp1p.tile([CH1_N, EB, 128],
                                   BF if use_tmode else F32, tag="tp1")
                    for e in range(EB):
                        g = bi * EB + e
                        if use_tmode:
                            nc.tensor.transpose(
                                p0[:, e], f_nat[:, g, CH0_LO:CH0_HI], id_sb)
                            nc.tensor.transpose(
                                p1[:, e], f_nat[:, g, CH1_LO:CH1_HI], id_sb)
                        else:
                            nc.tensor.matmul(
                                p0[:, e], f_nat[:, g, CH0_LO:CH0_HI],
                                id_sb, start=True, stop=True)
                            nc.tensor.matmul(
                                p1[:, e], f_nat[:, g, CH1_LO:CH1_HI],
                                id_sb, start=True, stop=True)
                    cols = slice(bi * EB * 128, (bi + 1) * EB * 128)
                    nc.scalar.copy(ft0[:, cols], p0[:])
                    nc.scalar.copy(ft1[:, cols], p1[:])

            # ---- main contraction + species weights ----
            gb = gbufp.tile([ND * NQ, BC], BF)
            NGRP = 4 if TRANSPOSE_MODE == "dma" else 8
            W = BC // NGRP
            for n in range(NGRP):
                cols = slice(n * W, (n + 1) * W)
                t_ps = tpsp.tile([ND * NQ, W], F32, tag="tps")
                for m in range(W // 512):
                    mc = slice(m * 512, (m + 1) * 512)
                    fc = slice(n * W + m * 512, n * W + (m + 1) * 512)
                    nc.tensor.matmul(t_ps[:, mc], u0_sb, ft0[:, fc],
                                     start=True, stop=False)
                    nc.tensor.matmul(t_ps[:, mc], u1_sb, ft1[:, fc],
                                     start=False, stop=True)
                nc.vector.tensor_mul(gb[:, cols], wexp_sb[:, cols], t_ps[:])

            # ---- segmented sum over q (PE): f[bc, D] ----
            f_ps = fpsp.tile([128, G, ND], F32)
            for g in range(G):
                nc.tensor.matmul(f_ps[:, g], gb[:, g * 128:(g + 1) * 128],
                                 sseg_sb, start=True, stop=True)
            f_sb = fsbp.tile([128, G, ND], BF)
            nc.scalar.copy(f_sb[:], f_ps[:])

            # ---- final linear (block-diag Wlin over channels) ----
            o_ps = opsp.tile([128, 128], F32, tag="ops")
            nc.tensor.matmul(o_ps[:, 0:G], bw0_sb, f_sb[:, :, 0],
                             start=True, stop=True)
            nc.tensor.matmul(
                o_ps[:, G:G + G * 3].rearrange("p (g i) -> p g i", g=G),
                bw1_sb, f_sb[:, :, 1:4], start=True, stop=True)

            # ---- output (contiguous; host unshuffles) ----
            o_sb = fsbp.tile([128, 128], F32)
            nc.vector.tensor_copy(o_sb[:], o_ps[:])
            nc.sync.dma_start(o_d, o_sb[:])

    nc.compile()
    return nc


def _get_nc():
    if "nc" not in _CACHE:
        _CACHE["nc"] = _build_nc()
    return _CACHE["nc"]


def kernel(node_feats, node_specie,
           U3_0, U2_0, U1_0, w3_0, w2_0, w1_0,
           U3_1, U2_1, U1_1, w3_1, w2_1, w1_1,
           Wlin0, Wlin1):
    from concourse.bass_utils import run_bass_kernel_spmd

    in_maps = _host_pack(node_feats, node_specie,
                         U3_0, U2_0, U1_0, w3_0, w2_0, w1_0,
                         U3_1, U2_1, U1_1, w3_1, w2_1, w1_1,
                         Wlin0, Wlin1)
    nc = _get_nc()
    res = run_bass_kernel_spmd(nc, in_maps, core_ids=list(range(N_CORES)))
    return _host_unpack(res.results).astype(np.float32)
